# revision 21
# baseline (speedup 1.0000x reference)
"""CommutatorConv2d kernel for Trainium2 (Bass/Tile), 8-core data-parallel.

Math: the reference's commutator/anticommutator conv reduces exactly to a
single-channel 3x3 conv on the channel-summed input xs = x.sum(axis=1).
Writing the conv's horizontal taps as shifted copies and folding them into
the vertical band matrices gives a SINGLE matmul stage:

    out[b] = V0 @ shiftR(xs) + V1 @ xs + V2 @ shiftL(xs) + bias
    V_k = a[k]*T + Bm   (T tridiagonal-ones, Bm tridiagonal from K row
                         sums, a[k] from K column sums)

and since the V stage is linear, xs stays SPLIT into partial sums, each
in its own zero-edged pad buffer with its own 3 shifted V matmuls into
one accumulating PSUM group -- V matmuls pipeline at ~107ns on PE, so
extra V groups are far cheaper than merge adds on the DVE critical path.

v9 (bf16 in, f32 out): the HWDGE alternates WHOLE DMAs between the two
queues, so pieces arrive sequentially in global issue order at ~360GB/s.
Order: [head(cmat+b1d1), b1d2, b1p1, b1p2, b0d1, b0p1, b0d2, b0p2] --
batch 1 first (its tail hides under b0 streaming), then b0 alternating
DVE-tree / PE-fold pieces so both engines track the stream; the last
piece is a PE piece (fastest post-processing).  Per batch: 2 pieces DVE-
treed (d1 -> own pad, d2 -> own pad [b0] or gpsimd-merged [b1]), 2
pieces PE-identity-folded into PSUM, one tensor_reduce -> reduce pad.
"""

import numpy as np

B, C, H, W = 16, 32, 128, 128
N_CORES = 8
B_LOC = B // N_CORES

CMCOLS = 4 * W + 2
PC = 8 * W  # piece cols
N_JUNK = 8

_PROGRAM = None
LAST_RESULTS = None


def _build_program():
    import concourse.mybir as mybir
    from concourse import bacc
    from concourse.bass import MemorySpace
    from concourse.tile import TileContext

    bf16 = mybir.dt.bfloat16
    f32 = mybir.dt.float32
    nc = bacc.Bacc(
        "TRN2", target_bir_lowering=False, debug=False, num_devices=N_CORES
    )

    ncols = CMCOLS + 2 * C * W
    xc_dram = nc.dram_tensor("xc", (H, ncols), bf16, kind="ExternalInput")
    out_dram = nc.dram_tensor("out", (B_LOC, H, W), f32, kind="ExternalOutput")

    xc_ap = xc_dram.ap()
    out_ap = out_dram.ap()

    # global arrival order (queues alternate whole DMAs):
    #  1:head(cmat+b1d1)  2:b1d2  3:b1p1  4:b1p2  5:b0p1  6:b0d1  7:b0p2  8:b0d2
    # sync carries slots 1,3,5,7; scalar carries 2,4,6,8.  The LAST piece
    # is a DVE-tree piece: its tail chain (tree -> V_t2 -> bias -> store)
    # skips the cast+V_r hops, which pre-run once b0p2's folds stop.
    HEADC = CMCOLS + PC
    cof = {}
    c = HEADC
    for nm in ("b1p1", "b0p1", "b0p2", "b1d2", "b1p2", "b0d1", "b0d2"):
        cof[nm] = c
        c += PC

    with TileContext(nc) as tc:
        with (
            tc.tile_pool(name="xpool", bufs=1) as xpool,
            tc.tile_pool(name="spool", bufs=1) as spool,
            tc.tile_pool(name="psum", bufs=1, space=MemorySpace.PSUM) as ppool,
        ):
            scratch = spool.tile([H, 5 * W], bf16, tag="scratch")
            nc.gpsimd.memset(scratch, 0.0)
            pads = {}
            for nm in ("xsp1t", "xsp1r", "xsp0t1", "xsp0t2", "xsp0r"):
                t = spool.tile([H, W + 2], bf16, name=nm, tag=nm)
                nc.gpsimd.memset(t, 0.0)
                pads[nm] = t

            # ---- input DMAs ----
            head = xpool.tile([H, HEADC], bf16, tag="head")
            nc.sync.dma_start(out=head, in_=xc_ap[:, 0:HEADC])
            cm_sb = head[:, 0:CMCOLS]
            i_sb = cm_sb[:, 3 * W : 4 * W]
            bias_sb = cm_sb[:, 4 * W : 4 * W + 2].bitcast(f32)
            b1d1 = head[:, CMCOLS:HEADC]

            tiles = {}
            for nm in ("b1p1", "b0p1", "b0p2"):  # sync slots 3,5,7
                t = xpool.tile([H, PC], bf16, name=nm, tag=nm)
                nc.sync.dma_start(out=t, in_=xc_ap[:, cof[nm] : cof[nm] + PC])
                tiles[nm] = t
            for nm in ("b1d2", "b1p2", "b0d1", "b0d2"):  # scalar 2,4,6,8
                t = xpool.tile([H, PC], bf16, name=nm, tag=nm)
                nc.scalar.dma_start(out=t, in_=xc_ap[:, cof[nm] : cof[nm] + PC])
                tiles[nm] = t
            tiles["b1d1"] = b1d1

            # ---- PE warmup ----
            junk_psum = ppool.tile([H, 4 * W], f32, tag="junk")
            for _ in range(N_JUNK):
                nc.tensor.matmul(
                    junk_psum,
                    scratch[:, 0:W],
                    scratch[:, W : 5 * W],
                    start=True,
                    stop=True,
                    skip_group_check=True,
                )

            psum = {
                1: ppool.tile([H, W], f32, name="ps1", tag="ps1"),
                0: ppool.tile([H, W], f32, name="ps0", tag="ps0"),
            }
            o_psum = {
                1: ppool.tile([H, W], f32, name="op1", tag="op1"),
                0: ppool.tile([H, W], f32, name="op0", tag="op0"),
            }

            def fold(b, p, start, stop):
                # N=128 matmuls pipeline at ~107ns, so 8 accumulating mms
                # produce the channel sum DIRECTLY in a [128,128] psum --
                # no 4-partial layout, no 690ns tensor_reduce afterwards
                for c in range(8):
                    nc.tensor.matmul(
                        psum[b],
                        i_sb,
                        p[:, c * W : (c + 1) * W],
                        start=(start and c == 0),
                        stop=(stop and c == 7),
                        skip_group_check=True,
                    )

            def tree(p, dst):
                # [128,1024] -> [128,128]; final add lands in dst pad
                nc.vector.tensor_add(p[:, : 4 * W], p[:, : 4 * W], p[:, 4 * W :])
                nc.vector.tensor_add(p[:, : 2 * W], p[:, : 2 * W], p[:, 2 * W : 4 * W])
                nc.vector.tensor_add(dst, p[:, :W], p[:, W : 2 * W])

            def cast(b, dst):
                # psum already holds xs; evac-cast on the idle ACT engine
                # (overlaps DVE trees and PE V matmuls)
                with nc.allow_low_precision("bf16 partials; gate is 2e-2"):
                    nc.scalar.copy(dst, psum[b])

            def vmms(b, xsp, start, stop):
                for k in range(3):
                    nc.tensor.matmul(
                        o_psum[b],
                        cm_sb[:, k * W : (k + 1) * W],
                        xsp[:, k : k + W],
                        start=(start and k == 0),
                        stop=(stop and k == 2),
                        skip_group_check=True,
                    )

            # ---- b1: folds, trees, reduce, V, bias, store -- fully
            # emitted BEFORE b0's folds so b1's tail never queues behind
            # b0's straggler-blocked pieces on any engine ----
            fold(1, tiles["b1p1"], True, False)
            fold(1, tiles["b1p2"], False, True)
            tree(tiles["b1d1"], tiles["b1d1"][:, 0:W])
            tree(tiles["b1d2"], tiles["b1d2"][:, 0:W])
            nc.gpsimd.tensor_add(
                pads["xsp1t"][:, 1 : W + 1],
                tiles["b1d1"][:, 0:W],
                tiles["b1d2"][:, 0:W],
            )
            cast(1, pads["xsp1r"][:, 1 : W + 1])
            vmms(1, pads["xsp1t"], True, False)
            vmms(1, pads["xsp1r"], False, True)
            osb1 = spool.tile([H, W], f32, tag="osb1")
            nc.scalar.add(osb1, o_psum[1], add=bias_sb)
            nc.sync.dma_start(out=out_ap[1, 0 : H // 2, :], in_=osb1[0 : H // 2, :])
            nc.scalar.dma_start(out=out_ap[1, H // 2 :, :], in_=osb1[H // 2 :, :])

            # ---- b0: folds + cast + their V groups pre-run; the tail is
            # only tree(b0d2) -> V_t2 -> bias -> store ----
            fold(0, tiles["b0p1"], True, False)
            fold(0, tiles["b0p2"], False, True)
            tree(tiles["b0d1"], pads["xsp0t1"][:, 1 : W + 1])
            cast(0, pads["xsp0r"][:, 1 : W + 1])
            vmms(0, pads["xsp0t1"], True, False)
            vmms(0, pads["xsp0r"], False, False)
            tree(tiles["b0d2"], pads["xsp0t2"][:, 1 : W + 1])
            vmms(0, pads["xsp0t2"], False, True)
            osb0 = spool.tile([H, W], f32, tag="osb0")
            nc.scalar.add(osb0, o_psum[0], add=bias_sb)
            nc.sync.dma_start(out=out_ap[0, 0 : H // 2, :], in_=osb0[0 : H // 2, :])
            nc.scalar.dma_start(out=out_ap[0, H // 2 :, :], in_=osb0[H // 2 :, :])

    nc.compile()
    return nc


def _get_program():
    global _PROGRAM
    if _PROGRAM is None:
        _PROGRAM = _build_program()
    return _PROGRAM


def _build_consts(K, bias, lambda_c, lambda_a):
    import ml_dtypes

    K = np.asarray(K, np.float32)
    lc = float(np.asarray(lambda_c))
    la = float(np.asarray(lambda_a))
    a = (lc + la) * K.sum(axis=0)  # column sums -> horizontal taps
    b = (la - lc) * K.sum(axis=1)  # row sums -> vertical taps
    eye = np.eye(H, dtype=np.float32)
    up = np.eye(H, k=1, dtype=np.float32)
    dn = np.eye(H, k=-1, dtype=np.float32)
    T = eye + up + dn
    Bm = b[1] * eye + b[2] * up + b[0] * dn
    vs = [np.ascontiguousarray((a[k] * T + Bm).T) for k in range(3)]
    cm = np.concatenate(vs + [eye], axis=1)
    cm16 = cm.astype(ml_dtypes.bfloat16)
    bias_col = np.full(
        (H, 1), np.asarray(bias, np.float32).reshape(-1)[0], np.float32
    )
    bias_bits = bias_col.view(np.uint16).view(ml_dtypes.bfloat16)  # [H, 2]
    return np.concatenate([cm16, bias_bits], axis=1)


def kernel(x, K, bias, lambda_c, lambda_a, _trace=False):
    global LAST_RESULTS
    import ml_dtypes
    from concourse.bass_utils import run_bass_kernel_spmd

    x = np.asarray(x, np.float32)
    cmb = _build_consts(K, bias, lambda_c, lambda_a)
    nc = _get_program()

    in_maps = []
    for core in range(N_CORES):
        shard = x[core * B_LOC : (core + 1) * B_LOC]  # [2, C, H, W]
        st = shard.transpose(2, 0, 1, 3).astype(ml_dtypes.bfloat16)  # [H,2,C,W]
        blocks = [
            cmb,
            st[:, 1, 0:8].reshape(H, PC),     # b1d1 (in head)
            st[:, 1, 16:24].reshape(H, PC),   # b1p1
            st[:, 0, 16:24].reshape(H, PC),   # b0p1
            st[:, 0, 24:32].reshape(H, PC),   # b0p2
            st[:, 1, 8:16].reshape(H, PC),    # b1d2
            st[:, 1, 24:32].reshape(H, PC),   # b1p2
            st[:, 0, 0:8].reshape(H, PC),     # b0d1
            st[:, 0, 8:16].reshape(H, PC),    # b0d2
        ]
        xc = np.concatenate(blocks, axis=1)
        in_maps.append({"xc": np.ascontiguousarray(xc)})

    res = run_bass_kernel_spmd(
        nc, in_maps, core_ids=list(range(N_CORES)), trace=_trace
    )
    LAST_RESULTS = res
    out = np.concatenate([np.asarray(r["out"]) for r in res.results], axis=0)
    return out.reshape(B, 1, H, W).astype(np.float32, copy=False)


# revision 23
# speedup vs baseline: 1.0075x; 1.0075x over previous
"""CommutatorConv2d kernel for Trainium2 (Bass/Tile), 8-core data-parallel.

Math: the reference's commutator/anticommutator conv reduces exactly to a
single-channel 3x3 conv on the channel-summed input xs = x.sum(axis=1).
Writing the conv's horizontal taps as shifted copies and folding them into
the vertical band matrices gives a SINGLE matmul stage:

    out[b] = V0 @ shiftR(xs) + V1 @ xs + V2 @ shiftL(xs) + bias
    V_k = a[k]*T + Bm   (T tridiagonal-ones, Bm tridiagonal from K row
                         sums, a[k] from K column sums)

and since the V stage is linear, xs stays SPLIT into partial sums, each
in its own zero-edged pad buffer with its own 3 shifted V matmuls into
one accumulating PSUM group -- V matmuls pipeline at ~107ns on PE, so
extra V groups are far cheaper than merge adds on the DVE critical path.

v9 (bf16 in, f32 out): the HWDGE alternates WHOLE DMAs between the two
queues, so pieces arrive sequentially in global issue order at ~360GB/s.
Order: [head(cmat+b1d1), b1d2, b1p1, b1p2, b0d1, b0p1, b0d2, b0p2] --
batch 1 first (its tail hides under b0 streaming), then b0 alternating
DVE-tree / PE-fold pieces so both engines track the stream; the last
piece is a PE piece (fastest post-processing).  Per batch: 2 pieces DVE-
treed (d1 -> own pad, d2 -> own pad [b0] or gpsimd-merged [b1]), 2
pieces PE-identity-folded into PSUM, one tensor_reduce -> reduce pad.
"""

import numpy as np

B, C, H, W = 16, 32, 128, 128
N_CORES = 8
B_LOC = B // N_CORES

CMCOLS = 4 * W + 2
PC = 8 * W  # piece cols
N_JUNK = 8

_PROGRAM = None
LAST_RESULTS = None


def _build_program():
    import concourse.mybir as mybir
    from concourse import bacc
    from concourse.bass import MemorySpace
    from concourse.tile import TileContext

    bf16 = mybir.dt.bfloat16
    f32 = mybir.dt.float32
    nc = bacc.Bacc(
        "TRN2", target_bir_lowering=False, debug=False, num_devices=N_CORES
    )

    ncols = CMCOLS + 2 * C * W
    xc_dram = nc.dram_tensor("xc", (H, ncols), bf16, kind="ExternalInput")
    out_dram = nc.dram_tensor("out", (B_LOC, H, W), f32, kind="ExternalOutput")

    xc_ap = xc_dram.ap()
    out_ap = out_dram.ap()

    # global arrival order (queues alternate whole DMAs):
    #  1:head(cmat+b1d1)  2:b1d2  3:b1p1  4:b1p2  5:b0p1  6:b0d1  7:b0p2  8:b0d2
    # sync carries slots 1,3,5,7; scalar carries 2,4,6,8.  The LAST piece
    # is a DVE-tree piece: its tail chain (tree -> V_t2 -> bias -> store)
    # skips the cast+V_r hops, which pre-run once b0p2's folds stop.
    HEADC = CMCOLS + PC
    cof = {}
    c = HEADC
    for nm in ("b1p1", "b0p1", "b0p2", "b1d2", "b1p2", "b0d1", "b0d2"):
        cof[nm] = c
        c += PC

    with TileContext(nc) as tc:
        with (
            tc.tile_pool(name="xpool", bufs=1) as xpool,
            tc.tile_pool(name="spool", bufs=1) as spool,
            tc.tile_pool(name="psum", bufs=1, space=MemorySpace.PSUM) as ppool,
        ):
            scratch = spool.tile([H, 5 * W], bf16, tag="scratch")
            nc.gpsimd.memset(scratch, 0.0)
            pads = {}
            for nm in ("xsp1t", "xsp1r", "xsp0t1", "xsp0t2", "xsp0r"):
                t = spool.tile([H, W + 2], bf16, name=nm, tag=nm)
                nc.gpsimd.memset(t, 0.0)
                pads[nm] = t

            # ---- input DMAs ----
            head = xpool.tile([H, HEADC], bf16, tag="head")
            nc.sync.dma_start(out=head, in_=xc_ap[:, 0:HEADC])
            cm_sb = head[:, 0:CMCOLS]
            i_sb = cm_sb[:, 3 * W : 4 * W]
            bias_sb = cm_sb[:, 4 * W : 4 * W + 2].bitcast(f32)
            b1d1 = head[:, CMCOLS:HEADC]

            tiles = {}
            for nm in ("b1p1", "b0p1", "b0p2"):  # sync slots 3,5,7
                t = xpool.tile([H, PC], bf16, name=nm, tag=nm)
                nc.sync.dma_start(out=t, in_=xc_ap[:, cof[nm] : cof[nm] + PC])
                tiles[nm] = t
            for nm in ("b1d2", "b1p2", "b0d1", "b0d2"):  # scalar 2,4,6,8
                t = xpool.tile([H, PC], bf16, name=nm, tag=nm)
                nc.scalar.dma_start(out=t, in_=xc_ap[:, cof[nm] : cof[nm] + PC])
                tiles[nm] = t
            tiles["b1d1"] = b1d1

            # ---- PE warmup ----
            junk_psum = ppool.tile([H, 4 * W], f32, tag="junk")
            for _ in range(N_JUNK):
                nc.tensor.matmul(
                    junk_psum,
                    scratch[:, 0:W],
                    scratch[:, W : 5 * W],
                    start=True,
                    stop=True,
                    skip_group_check=True,
                )

            psum = {
                1: ppool.tile([H, W], f32, name="ps1", tag="ps1"),
                0: ppool.tile([H, W], f32, name="ps0", tag="ps0"),
            }
            o_psum = {
                1: ppool.tile([H, W], f32, name="op1", tag="op1"),
                0: ppool.tile([H, W], f32, name="op0", tag="op0"),
            }

            def fold(b, p, start, stop):
                # N=128 matmuls pipeline at ~107ns, so 8 accumulating mms
                # produce the channel sum DIRECTLY in a [128,128] psum --
                # no 4-partial layout, no 690ns tensor_reduce afterwards
                for c in range(8):
                    nc.tensor.matmul(
                        psum[b],
                        i_sb,
                        p[:, c * W : (c + 1) * W],
                        start=(start and c == 0),
                        stop=(stop and c == 7),
                        skip_group_check=True,
                    )

            def tree(p, dst):
                # [128,1024] -> [128,128]; final add lands in dst pad
                nc.vector.tensor_add(p[:, : 4 * W], p[:, : 4 * W], p[:, 4 * W :])
                nc.vector.tensor_add(p[:, : 2 * W], p[:, : 2 * W], p[:, 2 * W : 4 * W])
                nc.vector.tensor_add(dst, p[:, :W], p[:, W : 2 * W])

            def cast(b, dst):
                # psum already holds xs; evac-cast it to a bf16 pad.
                # b1's cast runs on DVE: ACT's compute ops sit behind its
                # queued DMA instrs (queue backpressure) until ~16us, far
                # too late for b1's tail.  b0's cast is needed late anyway
                # and stays on ACT, overlapping DVE's final tree.
                with nc.allow_low_precision("bf16 partials; gate is 2e-2"):
                    if b == 1:
                        nc.vector.tensor_copy(dst, psum[b])
                    else:
                        nc.scalar.copy(dst, psum[b])

            def vmms(b, xsp, start, stop):
                for k in range(3):
                    nc.tensor.matmul(
                        o_psum[b],
                        cm_sb[:, k * W : (k + 1) * W],
                        xsp[:, k : k + W],
                        start=(start and k == 0),
                        stop=(stop and k == 2),
                        skip_group_check=True,
                    )

            # ---- b1: folds, trees, reduce, V, bias, store -- fully
            # emitted BEFORE b0's folds so b1's tail never queues behind
            # b0's straggler-blocked pieces on any engine ----
            fold(1, tiles["b1p1"], True, False)
            fold(1, tiles["b1p2"], False, True)
            tree(tiles["b1d1"], tiles["b1d1"][:, 0:W])
            tree(tiles["b1d2"], tiles["b1d2"][:, 0:W])
            nc.gpsimd.tensor_add(
                pads["xsp1t"][:, 1 : W + 1],
                tiles["b1d1"][:, 0:W],
                tiles["b1d2"][:, 0:W],
            )
            cast(1, pads["xsp1r"][:, 1 : W + 1])
            vmms(1, pads["xsp1t"], True, False)
            vmms(1, pads["xsp1r"], False, True)

            # ---- b0 folds + first tree; b1's bias (on DVE, between b0's
            # trees) + store launch as early as possible so b1's store
            # descriptor-gen doesn't serialize ahead of b0's ----
            fold(0, tiles["b0p1"], True, False)
            fold(0, tiles["b0p2"], False, True)
            tree(tiles["b0d1"], pads["xsp0t1"][:, 1 : W + 1])
            osb1 = spool.tile([H, W], f32, tag="osb1")
            nc.vector.tensor_scalar_add(osb1, o_psum[1], bias_sb)
            nc.sync.dma_start(out=out_ap[1, 0 : H // 2, :], in_=osb1[0 : H // 2, :])
            nc.scalar.dma_start(out=out_ap[1, H // 2 :, :], in_=osb1[H // 2 :, :])

            # ---- b0 tail: cast + V_t1/V_r pre-run; then only
            # tree(b0d2) -> V_t2 -> bias -> store remains ----
            cast(0, pads["xsp0r"][:, 1 : W + 1])
            vmms(0, pads["xsp0t1"], True, False)
            vmms(0, pads["xsp0r"], False, False)
            tree(tiles["b0d2"], pads["xsp0t2"][:, 1 : W + 1])
            vmms(0, pads["xsp0t2"], False, True)
            osb0 = spool.tile([H, W], f32, tag="osb0")
            nc.scalar.add(osb0, o_psum[0], add=bias_sb)
            nc.sync.dma_start(out=out_ap[0, 0 : H // 2, :], in_=osb0[0 : H // 2, :])
            nc.scalar.dma_start(out=out_ap[0, H // 2 :, :], in_=osb0[H // 2 :, :])

    nc.compile()
    return nc


def _get_program():
    global _PROGRAM
    if _PROGRAM is None:
        _PROGRAM = _build_program()
    return _PROGRAM


def _build_consts(K, bias, lambda_c, lambda_a):
    import ml_dtypes

    K = np.asarray(K, np.float32)
    lc = float(np.asarray(lambda_c))
    la = float(np.asarray(lambda_a))
    a = (lc + la) * K.sum(axis=0)  # column sums -> horizontal taps
    b = (la - lc) * K.sum(axis=1)  # row sums -> vertical taps
    eye = np.eye(H, dtype=np.float32)
    up = np.eye(H, k=1, dtype=np.float32)
    dn = np.eye(H, k=-1, dtype=np.float32)
    T = eye + up + dn
    Bm = b[1] * eye + b[2] * up + b[0] * dn
    vs = [np.ascontiguousarray((a[k] * T + Bm).T) for k in range(3)]
    cm = np.concatenate(vs + [eye], axis=1)
    cm16 = cm.astype(ml_dtypes.bfloat16)
    bias_col = np.full(
        (H, 1), np.asarray(bias, np.float32).reshape(-1)[0], np.float32
    )
    bias_bits = bias_col.view(np.uint16).view(ml_dtypes.bfloat16)  # [H, 2]
    return np.concatenate([cm16, bias_bits], axis=1)


def kernel(x, K, bias, lambda_c, lambda_a, _trace=False):
    global LAST_RESULTS
    import ml_dtypes
    from concourse.bass_utils import run_bass_kernel_spmd

    x = np.asarray(x, np.float32)
    cmb = _build_consts(K, bias, lambda_c, lambda_a)
    nc = _get_program()

    in_maps = []
    for core in range(N_CORES):
        shard = x[core * B_LOC : (core + 1) * B_LOC]  # [2, C, H, W]
        st = shard.transpose(2, 0, 1, 3).astype(ml_dtypes.bfloat16)  # [H,2,C,W]
        blocks = [
            cmb,
            st[:, 1, 0:8].reshape(H, PC),     # b1d1 (in head)
            st[:, 1, 16:24].reshape(H, PC),   # b1p1
            st[:, 0, 16:24].reshape(H, PC),   # b0p1
            st[:, 0, 24:32].reshape(H, PC),   # b0p2
            st[:, 1, 8:16].reshape(H, PC),    # b1d2
            st[:, 1, 24:32].reshape(H, PC),   # b1p2
            st[:, 0, 0:8].reshape(H, PC),     # b0d1
            st[:, 0, 8:16].reshape(H, PC),    # b0d2
        ]
        xc = np.concatenate(blocks, axis=1)
        in_maps.append({"xc": np.ascontiguousarray(xc)})

    res = run_bass_kernel_spmd(
        nc, in_maps, core_ids=list(range(N_CORES)), trace=_trace
    )
    LAST_RESULTS = res
    out = np.concatenate([np.asarray(r["out"]) for r in res.results], axis=0)
    return out.reshape(B, 1, H, W).astype(np.float32, copy=False)


# revision 24
# speedup vs baseline: 1.0175x; 1.0099x over previous
"""CommutatorConv2d kernel for Trainium2 (Bass/Tile), 8-core data-parallel.

Math: the reference's commutator/anticommutator conv reduces exactly to a
single-channel 3x3 conv on the channel-summed input xs = x.sum(axis=1).
Writing the conv's horizontal taps as shifted copies and folding them into
the vertical band matrices gives a SINGLE matmul stage:

    out[b] = V0 @ shiftR(xs) + V1 @ xs + V2 @ shiftL(xs) + bias
    V_k = a[k]*T + Bm   (T tridiagonal-ones, Bm tridiagonal from K row
                         sums, a[k] from K column sums)

and since the V stage is linear, xs stays SPLIT into partial sums, each
in its own zero-edged pad buffer with its own 3 shifted V matmuls into
one accumulating PSUM group -- V matmuls pipeline at ~107ns on PE, so
extra V groups are far cheaper than merge adds on the DVE critical path.

v9 (bf16 in, f32 out): the HWDGE alternates WHOLE DMAs between the two
queues, so pieces arrive sequentially in global issue order at ~360GB/s.
Order: [head(cmat+b1d1), b1d2, b1p1, b1p2, b0d1, b0p1, b0d2, b0p2] --
batch 1 first (its tail hides under b0 streaming), then b0 alternating
DVE-tree / PE-fold pieces so both engines track the stream; the last
piece is a PE piece (fastest post-processing).  Per batch: 2 pieces DVE-
treed (d1 -> own pad, d2 -> own pad [b0] or gpsimd-merged [b1]), 2
pieces PE-identity-folded into PSUM, one tensor_reduce -> reduce pad.
"""

import numpy as np

B, C, H, W = 16, 32, 128, 128
N_CORES = 8
B_LOC = B // N_CORES

CMCOLS = 4 * W + 2
PC = 8 * W  # piece cols
N_JUNK = 8

_PROGRAM = None
LAST_RESULTS = None


def _build_program():
    import concourse.mybir as mybir
    from concourse import bacc
    from concourse.bass import MemorySpace
    from concourse.tile import TileContext

    bf16 = mybir.dt.bfloat16
    f32 = mybir.dt.float32
    nc = bacc.Bacc(
        "TRN2", target_bir_lowering=False, debug=False, num_devices=N_CORES
    )

    ncols = CMCOLS + 2 * C * W
    xc_dram = nc.dram_tensor("xc", (H, ncols), bf16, kind="ExternalInput")
    out_dram = nc.dram_tensor("out", (B_LOC, H, W), f32, kind="ExternalOutput")

    xc_ap = xc_dram.ap()
    out_ap = out_dram.ap()

    # global arrival order (queues alternate whole DMAs):
    #  1:head(cmat+b1d1)  2:b1d2  3:b1p1  4:b1p2  5:b0d1  6:b0p1  7:b0d2  8:b0p2
    # sync carries slots 1,3,5,7; scalar carries 2,4,6,8
    HEADC = CMCOLS + PC
    cof = {}
    c = HEADC
    for nm in ("b1p1", "b0d1", "b0d2", "b1d2", "b1p2", "b0p1", "b0p2"):
        cof[nm] = c
        c += PC

    with TileContext(nc) as tc:
        with (
            tc.tile_pool(name="xpool", bufs=1) as xpool,
            tc.tile_pool(name="spool", bufs=1) as spool,
            tc.tile_pool(name="psum", bufs=1, space=MemorySpace.PSUM) as ppool,
        ):
            scratch = spool.tile([H, 5 * W], bf16, tag="scratch")
            nc.gpsimd.memset(scratch, 0.0)
            pads = {}
            for nm in ("xsp1t", "xsp1r", "xsp0t1", "xsp0t2", "xsp0r"):
                t = spool.tile([H, W + 2], bf16, name=nm, tag=nm)
                nc.gpsimd.memset(t, 0.0)
                pads[nm] = t

            # ---- input DMAs ----
            head = xpool.tile([H, HEADC], bf16, tag="head")
            nc.sync.dma_start(out=head, in_=xc_ap[:, 0:HEADC])
            cm_sb = head[:, 0:CMCOLS]
            i_sb = cm_sb[:, 3 * W : 4 * W]
            bias_sb = cm_sb[:, 4 * W : 4 * W + 2].bitcast(f32)
            b1d1 = head[:, CMCOLS:HEADC]

            tiles = {}
            for nm in ("b1p1", "b0d1", "b0d2"):  # sync slots 3,5,7
                t = xpool.tile([H, PC], bf16, name=nm, tag=nm)
                nc.sync.dma_start(out=t, in_=xc_ap[:, cof[nm] : cof[nm] + PC])
                tiles[nm] = t
            for nm in ("b1d2", "b1p2", "b0p1", "b0p2"):  # scalar 2,4,6,8
                t = xpool.tile([H, PC], bf16, name=nm, tag=nm)
                nc.scalar.dma_start(out=t, in_=xc_ap[:, cof[nm] : cof[nm] + PC])
                tiles[nm] = t
            tiles["b1d1"] = b1d1

            # ---- PE warmup ----
            junk_psum = ppool.tile([H, 4 * W], f32, tag="junk")
            for _ in range(N_JUNK):
                nc.tensor.matmul(
                    junk_psum,
                    scratch[:, 0:W],
                    scratch[:, W : 5 * W],
                    start=True,
                    stop=True,
                    skip_group_check=True,
                )

            psum = {
                1: ppool.tile([H, W], f32, name="ps1", tag="ps1"),
                0: ppool.tile([H, W], f32, name="ps0", tag="ps0"),
            }
            o_psum = {
                1: ppool.tile([H, W], f32, name="op1", tag="op1"),
                0: ppool.tile([H, W], f32, name="op0", tag="op0"),
            }

            def fold(b, p, start, stop):
                # N=128 matmuls pipeline at ~107ns, so 8 accumulating mms
                # produce the channel sum DIRECTLY in a [128,128] psum --
                # no 4-partial layout, no 690ns tensor_reduce afterwards
                for c in range(8):
                    nc.tensor.matmul(
                        psum[b],
                        i_sb,
                        p[:, c * W : (c + 1) * W],
                        start=(start and c == 0),
                        stop=(stop and c == 7),
                        skip_group_check=True,
                    )

            def tree(p, dst):
                # [128,1024] -> [128,128]; final add lands in dst pad
                nc.vector.tensor_add(p[:, : 4 * W], p[:, : 4 * W], p[:, 4 * W :])
                nc.vector.tensor_add(p[:, : 2 * W], p[:, : 2 * W], p[:, 2 * W : 4 * W])
                nc.vector.tensor_add(dst, p[:, :W], p[:, W : 2 * W])

            def cast(b, dst):
                # psum already holds xs; evac-cast on the idle ACT engine
                # (overlaps DVE trees and PE V matmuls)
                with nc.allow_low_precision("bf16 partials; gate is 2e-2"):
                    nc.scalar.copy(dst, psum[b])

            def vmms(b, xsp, start, stop):
                for k in range(3):
                    nc.tensor.matmul(
                        o_psum[b],
                        cm_sb[:, k * W : (k + 1) * W],
                        xsp[:, k : k + W],
                        start=(start and k == 0),
                        stop=(stop and k == 2),
                        skip_group_check=True,
                    )

            # ---- b1: folds, trees, reduce, V, bias, store -- fully
            # emitted BEFORE b0's folds so b1's tail never queues behind
            # b0's straggler-blocked pieces on any engine ----
            fold(1, tiles["b1p1"], True, False)
            fold(1, tiles["b1p2"], False, True)
            tree(tiles["b1d1"], tiles["b1d1"][:, 0:W])
            tree(tiles["b1d2"], tiles["b1d2"][:, 0:W])
            nc.gpsimd.tensor_add(
                pads["xsp1t"][:, 1 : W + 1],
                tiles["b1d1"][:, 0:W],
                tiles["b1d2"][:, 0:W],
            )
            cast(1, pads["xsp1r"][:, 1 : W + 1])
            vmms(1, pads["xsp1t"], True, False)
            vmms(1, pads["xsp1r"], False, True)
            osb1 = spool.tile([H, W], f32, tag="osb1")
            nc.scalar.add(osb1, o_psum[1], add=bias_sb)
            nc.sync.dma_start(out=out_ap[1, 0 : H // 2, :], in_=osb1[0 : H // 2, :])
            nc.scalar.dma_start(out=out_ap[1, H // 2 :, :], in_=osb1[H // 2 :, :])

            # ---- b0 ----
            fold(0, tiles["b0p1"], True, False)
            fold(0, tiles["b0p2"], False, True)
            tree(tiles["b0d1"], pads["xsp0t1"][:, 1 : W + 1])
            tree(tiles["b0d2"], pads["xsp0t2"][:, 1 : W + 1])
            cast(0, pads["xsp0r"][:, 1 : W + 1])
            vmms(0, pads["xsp0t1"], True, False)
            vmms(0, pads["xsp0t2"], False, False)
            vmms(0, pads["xsp0r"], False, True)
            osb0 = spool.tile([H, W], f32, tag="osb0")
            nc.scalar.add(osb0, o_psum[0], add=bias_sb)
            nc.sync.dma_start(out=out_ap[0, 0 : H // 2, :], in_=osb0[0 : H // 2, :])
            nc.scalar.dma_start(out=out_ap[0, H // 2 :, :], in_=osb0[H // 2 :, :])

    nc.compile()
    return nc


def _get_program():
    global _PROGRAM
    if _PROGRAM is None:
        _PROGRAM = _build_program()
    return _PROGRAM


def _build_consts(K, bias, lambda_c, lambda_a):
    import ml_dtypes

    K = np.asarray(K, np.float32)
    lc = float(np.asarray(lambda_c))
    la = float(np.asarray(lambda_a))
    a = (lc + la) * K.sum(axis=0)  # column sums -> horizontal taps
    b = (la - lc) * K.sum(axis=1)  # row sums -> vertical taps
    eye = np.eye(H, dtype=np.float32)
    up = np.eye(H, k=1, dtype=np.float32)
    dn = np.eye(H, k=-1, dtype=np.float32)
    T = eye + up + dn
    Bm = b[1] * eye + b[2] * up + b[0] * dn
    vs = [np.ascontiguousarray((a[k] * T + Bm).T) for k in range(3)]
    cm = np.concatenate(vs + [eye], axis=1)
    cm16 = cm.astype(ml_dtypes.bfloat16)
    bias_col = np.full(
        (H, 1), np.asarray(bias, np.float32).reshape(-1)[0], np.float32
    )
    bias_bits = bias_col.view(np.uint16).view(ml_dtypes.bfloat16)  # [H, 2]
    return np.concatenate([cm16, bias_bits], axis=1)


def kernel(x, K, bias, lambda_c, lambda_a, _trace=False):
    global LAST_RESULTS
    import ml_dtypes
    from concourse.bass_utils import run_bass_kernel_spmd

    x = np.asarray(x, np.float32)
    cmb = _build_consts(K, bias, lambda_c, lambda_a)
    nc = _get_program()

    in_maps = []
    for core in range(N_CORES):
        shard = x[core * B_LOC : (core + 1) * B_LOC]  # [2, C, H, W]
        st = shard.transpose(2, 0, 1, 3).astype(ml_dtypes.bfloat16)  # [H,2,C,W]
        blocks = [
            cmb,
            st[:, 1, 0:8].reshape(H, PC),     # b1d1 (in head)
            st[:, 1, 16:24].reshape(H, PC),   # b1p1
            st[:, 0, 0:8].reshape(H, PC),     # b0d1
            st[:, 0, 8:16].reshape(H, PC),    # b0d2
            st[:, 1, 8:16].reshape(H, PC),    # b1d2
            st[:, 1, 24:32].reshape(H, PC),   # b1p2
            st[:, 0, 16:24].reshape(H, PC),   # b0p1
            st[:, 0, 24:32].reshape(H, PC),   # b0p2
        ]
        xc = np.concatenate(blocks, axis=1)
        in_maps.append({"xc": np.ascontiguousarray(xc)})

    res = run_bass_kernel_spmd(
        nc, in_maps, core_ids=list(range(N_CORES)), trace=_trace
    )
    LAST_RESULTS = res
    out = np.concatenate([np.asarray(r["out"]) for r in res.results], axis=0)
    return out.reshape(B, 1, H, W).astype(np.float32, copy=False)
